# revision 3
# baseline (speedup 1.0000x reference)
"""Trainium2 Bass kernel: 2-layer LSTM (B=1024, T=512, H=256) + linear head.

Data-parallel across 8 NeuronCores: each core runs the full sequential scan
for a 128-row batch shard. Host-side work is marshaling only: sharding,
weight transposes/permutation, folding the day-embedding into layer-0 input
weights, and one-hot encoding the integer day column.

Per-step device schedule (per layer): gates [B=128, 4H=1024] accumulate in
PSUM via float32r matmuls with N=512 (stationary = transposed state tiles,
moving = transposed weights); biases ride the matmuls (aug ones-row for
layer 0, a K=1 ones matmul for layer 1). Sigmoid covers one contiguous
[128, 768] span (gate columns permuted to [i f o g]), tanh two more ACT
instructions; cell/hidden updates on VectorE; the new hidden state is
re-transposed via two PE transposes + one DVE copy for the next step.
"""

import sys

import numpy as np

try:
    import concourse.bass as _probe  # noqa: F401
except ImportError:
    sys.path.insert(0, "/opt/trn_rl_repo")

B_FULL, T, D, H, P_OUT = 1024, 512, 64, 256, 14
N_CORES = 8
B = B_FULL // N_CORES  # 128 rows per core
G = 4 * H  # 1024 gate width
FA = 16  # augmented input rows: [val, onehot(day) x7, ones, pad x7]
CH = 64  # timesteps per aug SBUF chunk
NCH = T // CH

# Gate columns are permuted from PyTorch order [i f g o] to [i f o g] so one
# sigmoid instruction covers cols 0:768 and one tanh covers 768:1024.
_PERM = np.concatenate(
    [np.arange(0, 512), np.arange(768, 1024), np.arange(512, 768)]
)

_MODULE = None
LAST_RESULTS = None


def _build_module():
    from contextlib import ExitStack

    import concourse.mybir as mybir
    from concourse import bacc
    from concourse.masks import make_identity
    from concourse.tile import TileContext

    f32 = mybir.dt.float32
    f32r = mybir.dt.float32r
    Sig = mybir.ActivationFunctionType.Sigmoid
    Tanh = mybir.ActivationFunctionType.Tanh

    nc = bacc.Bacc()
    aug_d = nc.dram_tensor("aug", [FA, T * B], f32r, kind="ExternalInput")
    w0t_d = nc.dram_tensor("w0t", [FA, G], f32r, kind="ExternalInput")
    whh0t_d = nc.dram_tensor("whh0t", [H, G], f32r, kind="ExternalInput")
    wih1t_d = nc.dram_tensor("wih1t", [H, G], f32r, kind="ExternalInput")
    whh1t_d = nc.dram_tensor("whh1t", [H, G], f32r, kind="ExternalInput")
    b1_d = nc.dram_tensor("b1", [1, G], f32r, kind="ExternalInput")
    ones_d = nc.dram_tensor("ones", [1, B], f32r, kind="ExternalInput")
    wlint_d = nc.dram_tensor("wlint", [H, P_OUT], f32r, kind="ExternalInput")
    blin_d = nc.dram_tensor("blin", [1, P_OUT], f32r, kind="ExternalInput")
    out_d = nc.dram_tensor("out", [B, P_OUT], f32, kind="ExternalOutput")

    with TileContext(nc) as tc, ExitStack() as ctx:
        consts = ctx.enter_context(tc.tile_pool(name="consts", bufs=1))
        augp = ctx.enter_context(tc.tile_pool(name="augp", bufs=2))
        h0Tp = ctx.enter_context(tc.tile_pool(name="h0Tp", bufs=3))
        h1Tp = ctx.enter_context(tc.tile_pool(name="h1Tp", bufs=3))
        c0p = ctx.enter_context(tc.tile_pool(name="c0p", bufs=2))
        c1p = ctx.enter_context(tc.tile_pool(name="c1p", bufs=2))
        acts = ctx.enter_context(tc.tile_pool(name="acts", bufs=2))
        g0pp = ctx.enter_context(tc.tile_pool(name="g0pp", bufs=2, space="PSUM"))
        g1pp = ctx.enter_context(tc.tile_pool(name="g1pp", bufs=1, space="PSUM"))
        hTps = ctx.enter_context(tc.tile_pool(name="hTps", bufs=2, space="PSUM"))

        # --- constants to SBUF ---
        w0t_sb = consts.tile([FA, G], f32r, tag="w0t")
        nc.sync.dma_start(w0t_sb, w0t_d[:, :])
        whh0t_sb = consts.tile([128, 2 * G], f32r, tag="whh0t")
        wih1t_sb = consts.tile([128, 2 * G], f32r, tag="wih1t")
        whh1t_sb = consts.tile([128, 2 * G], f32r, tag="whh1t")
        for k in range(2):
            nc.sync.dma_start(
                whh0t_sb[:, k * G : (k + 1) * G], whh0t_d[k * 128 : (k + 1) * 128, :]
            )
            nc.sync.dma_start(
                wih1t_sb[:, k * G : (k + 1) * G], wih1t_d[k * 128 : (k + 1) * 128, :]
            )
            nc.sync.dma_start(
                whh1t_sb[:, k * G : (k + 1) * G], whh1t_d[k * 128 : (k + 1) * 128, :]
            )
        b1_sb = consts.tile([1, G], f32r, tag="b1")
        nc.sync.dma_start(b1_sb, b1_d[:, :])
        ones_sb = consts.tile([1, B], f32r, tag="ones")
        nc.sync.dma_start(ones_sb, ones_d[:, :])
        wlint_sb = consts.tile([128, 2 * P_OUT], f32r, tag="wlint")
        for k in range(2):
            nc.sync.dma_start(
                wlint_sb[:, k * P_OUT : (k + 1) * P_OUT],
                wlint_d[k * 128 : (k + 1) * 128, :],
            )
        blin_sb = consts.tile([1, P_OUT], f32r, tag="blin")
        nc.sync.dma_start(blin_sb, blin_d[:, :])
        ident = consts.tile([128, 128], f32, tag="ident")
        make_identity(nc, ident)

        aug_tiles = [None] * NCH

        def load_chunk(chi):
            tl = augp.tile([FA, CH * B], f32r, tag="augc")
            nc.sync.dma_start(tl, aug_d[:, chi * CH * B : (chi + 1) * CH * B])
            aug_tiles[chi] = tl

        load_chunk(0)

        mm = nc.tensor.matmul

        h0T = h1T = c0 = c1 = None

        for t in range(T):
            first = t == 0
            chi = t // CH
            if t % CH == 0 and chi + 1 < NCH:
                load_chunk(chi + 1)
            aug_sl = aug_tiles[chi][:, (t % CH) * B : (t % CH + 1) * B]  # [FA, B]

            # ---------------- layer 0 gates ----------------
            g0 = g0pp.tile([B, G], f32, tag="g0")
            for nb in range(2):
                sl = slice(nb * 512, (nb + 1) * 512)
                mm(g0[:, sl], aug_sl, w0t_sb[:, sl], start=True, stop=first)
                if not first:
                    for k in range(2):
                        mm(
                            g0[:, sl],
                            h0T[:, k * 128 : (k + 1) * 128],
                            whh0t_sb[:, k * G + nb * 512 : k * G + (nb + 1) * 512],
                            start=False,
                            stop=(k == 1),
                        )
            sig0 = acts.tile([B, 3 * H], f32, tag="sig0")
            nc.scalar.activation(sig0, g0[:, 0 : 3 * H], Sig)
            gt0 = acts.tile([B, H], f32, tag="gt0")
            nc.scalar.activation(gt0, g0[:, 3 * H : G], Tanh)
            c0n = c0p.tile([B, H], f32, tag="c0")
            if first:
                nc.vector.tensor_mul(c0n, sig0[:, 0:H], gt0)
            else:
                ig0 = acts.tile([B, H], f32, tag="ig0")
                nc.vector.tensor_mul(ig0, sig0[:, 0:H], gt0)
                fc0 = acts.tile([B, H], f32, tag="fc0")
                nc.vector.tensor_mul(fc0, sig0[:, H : 2 * H], c0)
                nc.vector.tensor_add(c0n, ig0, fc0)
            c0 = c0n
            tc0 = acts.tile([B, H], f32, tag="tc0")
            nc.scalar.activation(tc0, c0, Tanh)
            h0n = acts.tile([B, H], f32, tag="h0n")
            nc.vector.tensor_mul(h0n, sig0[:, 2 * H : 3 * H], tc0)
            h0T_ps = hTps.tile([128, H], f32, tag="htp")
            for k in range(2):
                nc.tensor.transpose(
                    h0T_ps[:, k * 128 : (k + 1) * 128],
                    h0n[:, k * 128 : (k + 1) * 128],
                    ident,
                )
            h0Tn = h0Tp.tile([128, H], f32r, tag="h0T")
            nc.vector.tensor_copy(h0Tn, h0T_ps)
            h0T = h0Tn

            # ---------------- layer 1 gates ----------------
            g1 = g1pp.tile([B, G], f32, tag="g1")
            for nb in range(2):
                sl = slice(nb * 512, (nb + 1) * 512)
                mm(g1[:, sl], ones_sb, b1_sb[:, sl], start=True, stop=False)
                for k in range(2):
                    mm(
                        g1[:, sl],
                        h0T[:, k * 128 : (k + 1) * 128],
                        wih1t_sb[:, k * G + nb * 512 : k * G + (nb + 1) * 512],
                        start=False,
                        stop=(first and k == 1),
                    )
                if not first:
                    for k in range(2):
                        mm(
                            g1[:, sl],
                            h1T[:, k * 128 : (k + 1) * 128],
                            whh1t_sb[:, k * G + nb * 512 : k * G + (nb + 1) * 512],
                            start=False,
                            stop=(k == 1),
                        )
            sig1 = acts.tile([B, 3 * H], f32, tag="sig1")
            nc.scalar.activation(sig1, g1[:, 0 : 3 * H], Sig)
            gt1 = acts.tile([B, H], f32, tag="gt1")
            nc.scalar.activation(gt1, g1[:, 3 * H : G], Tanh)
            c1n = c1p.tile([B, H], f32, tag="c1")
            if first:
                nc.vector.tensor_mul(c1n, sig1[:, 0:H], gt1)
            else:
                ig1 = acts.tile([B, H], f32, tag="ig1")
                nc.vector.tensor_mul(ig1, sig1[:, 0:H], gt1)
                fc1 = acts.tile([B, H], f32, tag="fc1")
                nc.vector.tensor_mul(fc1, sig1[:, H : 2 * H], c1)
                nc.vector.tensor_add(c1n, ig1, fc1)
            c1 = c1n
            tc1 = acts.tile([B, H], f32, tag="tc1")
            nc.scalar.activation(tc1, c1, Tanh)
            h1n = acts.tile([B, H], f32, tag="h1n")
            nc.vector.tensor_mul(h1n, sig1[:, 2 * H : 3 * H], tc1)
            h1T_ps = hTps.tile([128, H], f32, tag="htp")
            for k in range(2):
                nc.tensor.transpose(
                    h1T_ps[:, k * 128 : (k + 1) * 128],
                    h1n[:, k * 128 : (k + 1) * 128],
                    ident,
                )
            h1Tn = h1Tp.tile([128, H], f32r, tag="h1T")
            nc.vector.tensor_copy(h1Tn, h1T_ps)
            h1T = h1Tn

        # ------------- final linear: out = h1[T-1] @ Wlin.T + blin -------------
        outp = hTps.tile([B, P_OUT], f32, tag="htp")
        mm(outp, ones_sb, blin_sb, start=True, stop=False)
        for k in range(2):
            mm(
                outp,
                h1T[:, k * 128 : (k + 1) * 128],
                wlint_sb[:, k * P_OUT : (k + 1) * P_OUT],
                start=False,
                stop=(k == 1),
            )
        out_sb = consts.tile([B, P_OUT], f32, tag="outsb")
        nc.vector.tensor_copy(out_sb, outp)
        nc.sync.dma_start(out_d[:, :], out_sb)

    nc.finalize()
    return nc


def _get_module():
    global _MODULE
    if _MODULE is None:
        _MODULE = _build_module()
    return _MODULE


def kernel(**inputs):
    global LAST_RESULTS
    from concourse.bass_utils import run_bass_kernel_spmd

    f = lambda a: np.ascontiguousarray(np.asarray(a), dtype=np.float32)
    x = f(inputs["x"])
    emb = f(inputs["emb"])
    Wih0, Whh0 = f(inputs["Wih0"]), f(inputs["Whh0"])
    bih0, bhh0 = f(inputs["bih0"]), f(inputs["bhh0"])
    Wih1, Whh1 = f(inputs["Wih1"]), f(inputs["Whh1"])
    bih1, bhh1 = f(inputs["bih1"]), f(inputs["bhh1"])
    Wlin, blin = f(inputs["Wlin"]), f(inputs["blin"])

    # Fold embedding + biases into layer-0 input weights.
    w_val = Wih0[:, 0:1]  # [G, 1]
    M0 = Wih0[:, 1 : 1 + D] @ emb.T  # [G, 7]
    b0 = (bih0 + bhh0)[:, None]  # [G, 1]
    W0aug = np.concatenate(
        [w_val, M0, b0, np.zeros((G, FA - 9), np.float32)], axis=1
    )  # [G, FA]

    w0t = np.ascontiguousarray(W0aug[_PERM].T)  # [FA, G]
    whh0t = np.ascontiguousarray(Whh0[_PERM].T)  # [H, G]
    wih1t = np.ascontiguousarray(Wih1[_PERM].T)
    whh1t = np.ascontiguousarray(Whh1[_PERM].T)
    b1 = np.ascontiguousarray((bih1 + bhh1)[_PERM][None, :])  # [1, G]
    ones_row = np.ones((1, B), np.float32)
    wlint = np.ascontiguousarray(Wlin.T)  # [H, P_OUT]
    blin_r = np.ascontiguousarray(blin[None, :])  # [1, P_OUT]

    val = x[:, :, 0]  # [B_FULL, T]
    day = x[:, :, 1].astype(np.int32)  # [B_FULL, T]

    in_maps = []
    for c in range(N_CORES):
        sl = slice(c * B, (c + 1) * B)
        aug = np.zeros((FA, T, B), np.float32)
        aug[0] = val[sl].T
        dT = day[sl].T  # [T, B]
        for d in range(7):
            aug[1 + d] = dT == d
        aug[8] = 1.0
        in_maps.append(
            {
                "aug": np.ascontiguousarray(aug.reshape(FA, T * B)),
                "w0t": w0t,
                "whh0t": whh0t,
                "wih1t": wih1t,
                "whh1t": whh1t,
                "b1": b1,
                "ones": ones_row,
                "wlint": wlint,
                "blin": blin_r,
            }
        )

    res = run_bass_kernel_spmd(_get_module(), in_maps, core_ids=list(range(N_CORES)))
    LAST_RESULTS = res
    out = np.concatenate([r["out"] for r in res.results], axis=0)
    return np.ascontiguousarray(out, dtype=np.float32)


# revision 5
# speedup vs baseline: 1.7834x; 1.7834x over previous
"""Trainium2 Bass kernel: 2-layer LSTM (B=1024, T=512, H=256) + linear head.

Data-parallel across 8 NeuronCores: each core runs the full sequential scan
for a 128-row batch shard. Host-side work is marshaling only: sharding,
weight transposes/permutation, folding the day-embedding into layer-0 input
weights, and one-hot encoding the integer day column.

Device schedule (wavefront, one tick per timestep):
  tick t: [PE] transpose h1[t-2] | gates0[t] matmuls | gates1[t-1] matmuls
               (PSUM-bank interleaved, Whh1 terms last) | transpose h0[t]
          [ACT] cast h1T[t-2] | sigmoid0 | tanh g0 | tanh c0 | sigmoid1 | ...
          [DVE] cell/hidden updates + cast h0T[t]
Gates live in PSUM [B=128, 4H=1024] (two banks, matmuls N=512 float32r,
stationary = transposed state, moving = transposed weights). Biases ride
matmuls: layer 0 via the aug ones-row, layer 1 via a K=128 broadcast
matmul (e0 row-selector x full-row bias matrix). Gate columns are permuted
[i f g o] -> [i f o g] so one sigmoid instruction covers cols 0:768.
"""

import sys

import numpy as np

try:
    import concourse.bass as _probe  # noqa: F401
except ImportError:
    sys.path.insert(0, "/opt/trn_rl_repo")

B_FULL, T, D, H, P_OUT = 1024, 512, 64, 256, 14
N_CORES = 8
B = B_FULL // N_CORES  # 128 rows per core
G = 4 * H  # 1024 gate width
FA = 16  # augmented input rows: [val, onehot(day) x7, ones, pad x7]
CH = 64  # timesteps per aug SBUF chunk
NCH = T // CH

_PERM = np.concatenate(
    [np.arange(0, 512), np.arange(768, 1024), np.arange(512, 768)]
)

_MODULE = None
LAST_RESULTS = None


def _build_module():
    from contextlib import ExitStack

    import concourse.mybir as mybir
    from concourse import bacc
    from concourse.masks import make_identity
    from concourse.tile import TileContext

    f32 = mybir.dt.float32
    f32r = mybir.dt.float32r
    Sig = mybir.ActivationFunctionType.Sigmoid
    Tanh = mybir.ActivationFunctionType.Tanh

    nc = bacc.Bacc()
    aug_d = nc.dram_tensor("aug", [FA, T * B], f32r, kind="ExternalInput")
    z112_d = nc.dram_tensor("z112", [128 - FA, CH * B], f32r, kind="ExternalInput")
    w0t_d = nc.dram_tensor("w0t", [128, G], f32r, kind="ExternalInput")
    whh0t_d = nc.dram_tensor("whh0t", [H, G], f32r, kind="ExternalInput")
    wih1t_d = nc.dram_tensor("wih1t", [H, G], f32r, kind="ExternalInput")
    whh1t_d = nc.dram_tensor("whh1t", [H, G], f32r, kind="ExternalInput")
    e0_d = nc.dram_tensor("e0", [128, 128], f32r, kind="ExternalInput")
    b1f_d = nc.dram_tensor("b1f", [128, G], f32r, kind="ExternalInput")
    wlint_d = nc.dram_tensor("wlint", [H, P_OUT], f32r, kind="ExternalInput")
    blinf_d = nc.dram_tensor("blinf", [128, P_OUT], f32r, kind="ExternalInput")
    out_d = nc.dram_tensor("out", [B, P_OUT], f32, kind="ExternalOutput")

    with TileContext(nc) as tc, ExitStack() as ctx:
        consts = ctx.enter_context(tc.tile_pool(name="consts", bufs=1))
        h0Tp = ctx.enter_context(tc.tile_pool(name="h0Tp", bufs=3))
        h1Tp = ctx.enter_context(tc.tile_pool(name="h1Tp", bufs=3))
        c0p = ctx.enter_context(tc.tile_pool(name="c0p", bufs=2))
        c1p = ctx.enter_context(tc.tile_pool(name="c1p", bufs=2))
        acts = ctx.enter_context(tc.tile_pool(name="acts", bufs=2))
        g0pp = ctx.enter_context(tc.tile_pool(name="g0pp", bufs=1, space="PSUM"))
        g1pp = ctx.enter_context(tc.tile_pool(name="g1pp", bufs=1, space="PSUM"))
        hTps = ctx.enter_context(tc.tile_pool(name="hTps", bufs=2, space="PSUM"))

        # --- constants to SBUF ---
        w0t_sb = consts.tile([128, G], f32r, tag="w0t")
        nc.sync.dma_start(w0t_sb, w0t_d[:, :])
        whh0t_sb = consts.tile([128, 2 * G], f32r, tag="whh0t")
        wih1t_sb = consts.tile([128, 2 * G], f32r, tag="wih1t")
        whh1t_sb = consts.tile([128, 2 * G], f32r, tag="whh1t")
        for k in range(2):
            nc.sync.dma_start(
                whh0t_sb[:, k * G : (k + 1) * G], whh0t_d[k * 128 : (k + 1) * 128, :]
            )
            nc.sync.dma_start(
                wih1t_sb[:, k * G : (k + 1) * G], wih1t_d[k * 128 : (k + 1) * 128, :]
            )
            nc.sync.dma_start(
                whh1t_sb[:, k * G : (k + 1) * G], whh1t_d[k * 128 : (k + 1) * 128, :]
            )
        e0_sb = consts.tile([128, 128], f32r, tag="e0")
        nc.sync.dma_start(e0_sb, e0_d[:, :])
        b1f_sb = consts.tile([128, G], f32r, tag="b1f")
        nc.sync.dma_start(b1f_sb, b1f_d[:, :])
        wlint_sb = consts.tile([128, 2 * P_OUT], f32r, tag="wlint")
        for k in range(2):
            nc.sync.dma_start(
                wlint_sb[:, k * P_OUT : (k + 1) * P_OUT],
                wlint_d[k * 128 : (k + 1) * 128, :],
            )
        blinf_sb = consts.tile([128, P_OUT], f32r, tag="blinf")
        nc.sync.dma_start(blinf_sb, blinf_d[:, :])
        ident = consts.tile([128, 128], f32, tag="ident")
        make_identity(nc, ident)

        # Two persistent aug buffers (manual double-buffer). Rows FA:128 are
        # zeroed once so the aug matmul can run with K=128.
        aug_bufs = []
        for i in range(2):
            ab = consts.tile([128, CH * B], f32r, tag=f"augbuf{i}", name=f"augbuf{i}")
            nc.sync.dma_start(ab[FA:128, :], z112_d[:, :])
            aug_bufs.append(ab)

        def load_chunk(chi):
            nc.sync.dma_start(
                aug_bufs[chi % 2][0:FA, :],
                aug_d[:, chi * CH * B : (chi + 1) * CH * B],
            )

        load_chunk(0)
        load_chunk(1)

        mm = nc.tensor.matmul

        # state handles indexed by step (python refs; tiles come from pools)
        h0T = [None] * T
        h1T = [None] * T
        c0 = [None] * T
        c1 = [None] * T
        h0n = [None] * T
        h1n = [None] * T
        sig = [[None] * T, [None] * T]
        gt = [[None] * T, [None] * T]
        g0ps = [None] * T
        g1ps = [None] * T
        h1tps = [None] * T

        def emit_g0_mms(t):
            chi = t // CH
            if t % CH == 0:
                if chi + 2 < NCH:
                    load_chunk(chi + 2)
            aug_sl = aug_bufs[chi % 2][:, (t % CH) * B : (t % CH + 1) * B]
            g0 = g0pp.tile([B, G], f32, tag="g0", name=f"g0_{t}")
            g0ps[t] = g0
            bk = [slice(0, 512), slice(512, 1024)]
            if t == 0:
                for nb in range(2):
                    mm(g0[:, bk[nb]], aug_sl, w0t_sb[:, bk[nb]], start=True, stop=True)
                return
            hp = h0T[t - 1]
            for nb in range(2):
                mm(g0[:, bk[nb]], aug_sl, w0t_sb[:, bk[nb]], start=True, stop=False)
            for k in range(2):
                for nb in range(2):
                    mm(
                        g0[:, bk[nb]],
                        hp[:, k * 128 : (k + 1) * 128],
                        whh0t_sb[:, k * G + nb * 512 : k * G + (nb + 1) * 512],
                        start=False,
                        stop=(k == 1),
                    )

        def emit_g1_mms(t):
            g1 = g1pp.tile([B, G], f32, tag="g1", name=f"g1_{t}")
            g1ps[t] = g1
            bk = [slice(0, 512), slice(512, 1024)]
            for nb in range(2):
                mm(g1[:, bk[nb]], e0_sb, b1f_sb[:, bk[nb]], start=True, stop=False)
            hp = h0T[t]
            for k in range(2):
                for nb in range(2):
                    mm(
                        g1[:, bk[nb]],
                        hp[:, k * 128 : (k + 1) * 128],
                        wih1t_sb[:, k * G + nb * 512 : k * G + (nb + 1) * 512],
                        start=False,
                        stop=(t == 0 and k == 1),
                    )
            if t > 0:
                hq = h1T[t - 1]
                for k in range(2):
                    for nb in range(2):
                        mm(
                            g1[:, bk[nb]],
                            hq[:, k * 128 : (k + 1) * 128],
                            whh1t_sb[:, k * G + nb * 512 : k * G + (nb + 1) * 512],
                            start=False,
                            stop=(k == 1),
                        )

        def emit_chain(layer, t):
            gps = g0ps[t] if layer == 0 else g1ps[t]
            cp = c0p if layer == 0 else c1p
            cl = c0 if layer == 0 else c1
            hn = h0n if layer == 0 else h1n
            s = acts.tile([B, 3 * H], f32, tag=f"sig{layer}", name=f"sig{layer}_{t}")
            sig[layer][t] = s
            nc.scalar.activation(s, gps[:, 0 : 3 * H], Sig)
            g = acts.tile([B, H], f32, tag=f"gt{layer}", name=f"gt{layer}_{t}")
            gt[layer][t] = g
            nc.scalar.activation(g, gps[:, 3 * H : G], Tanh)
            cn = cp.tile([B, H], f32, tag=f"c{layer}", name=f"c{layer}_{t}")
            if t == 0:
                nc.vector.tensor_mul(cn, s[:, 0:H], g)
            else:
                ig = acts.tile([B, H], f32, tag=f"ig{layer}", name=f"ig{layer}_{t}")
                nc.vector.tensor_mul(ig, s[:, 0:H], g)
                fc = acts.tile([B, H], f32, tag=f"fc{layer}", name=f"fc{layer}_{t}")
                nc.vector.tensor_mul(fc, s[:, H : 2 * H], cl[t - 1])
                nc.vector.tensor_add(cn, ig, fc)
            cl[t] = cn
            tcx = acts.tile([B, H], f32, tag=f"tc{layer}", name=f"tc{layer}_{t}")
            nc.scalar.activation(tcx, cn, Tanh)
            h = acts.tile([B, H], f32, tag=f"hn{layer}", name=f"hn{layer}_{t}")
            nc.vector.tensor_mul(h, s[:, 2 * H : 3 * H], tcx)
            hn[t] = h

        def emit_h0_transp(t):
            ps = hTps.tile([128, 1024], f32, tag="htp", name=f"h0tp_{t}")
            nc.tensor.transpose(ps[:, 0:128], h0n[t][:, 0:128], ident)
            nc.tensor.transpose(ps[:, 512:640], h0n[t][:, 128:256], ident)
            hsb = h0Tp.tile([128, H], f32r, tag="h0T", name=f"h0T_{t}")
            nc.vector.tensor_copy(
                hsb.rearrange("p (b c) -> p b c", b=2),
                ps.rearrange("p (b c) -> p b c", b=2)[:, :, 0:128],
            )
            h0T[t] = hsb

        def emit_h1_transp(t):
            ps = hTps.tile([128, 1024], f32, tag="htp", name=f"h1tp_{t}")
            nc.tensor.transpose(ps[:, 0:128], h1n[t][:, 0:128], ident)
            nc.tensor.transpose(ps[:, 512:640], h1n[t][:, 128:256], ident)
            h1tps[t] = ps

        def emit_h1_cast(t):
            hsb = h1Tp.tile([128, H], f32r, tag="h1T", name=f"h1T_{t}")
            nc.scalar.copy(
                hsb.rearrange("p (b c) -> p b c", b=2),
                h1tps[t].rearrange("p (b c) -> p b c", b=2)[:, :, 0:128],
            )
            h1T[t] = hsb

        for tau in range(T + 2):
            if tau >= 2:
                emit_h1_transp(tau - 2)  # PE slot 0: h1n[tau-2] long ready
                emit_h1_cast(tau - 2)  # ACT, in its idle window at tick start
            if tau < T:
                emit_g0_mms(tau)
            if 1 <= tau <= T:
                emit_g1_mms(tau - 1)
            if tau < T:
                emit_chain(0, tau)
                emit_h0_transp(tau)
            if 1 <= tau <= T:
                emit_chain(1, tau - 1)

        # ------------- final linear: out = h1[T-1] @ Wlin.T + blin -------------
        outp = hTps.tile([B, P_OUT], f32, tag="htp", name="outp")
        mm(outp, e0_sb, blinf_sb, start=True, stop=False)
        hl = h1T[T - 1]
        for k in range(2):
            mm(
                outp,
                hl[:, k * 128 : (k + 1) * 128],
                wlint_sb[:, k * P_OUT : (k + 1) * P_OUT],
                start=False,
                stop=(k == 1),
            )
        out_sb = consts.tile([B, P_OUT], f32, tag="outsb")
        nc.vector.tensor_copy(out_sb, outp)
        nc.sync.dma_start(out_d[:, :], out_sb)

    nc.finalize()
    return nc


def _get_module():
    global _MODULE
    if _MODULE is None:
        _MODULE = _build_module()
    return _MODULE


def kernel(**inputs):
    global LAST_RESULTS
    from concourse.bass_utils import run_bass_kernel_spmd

    f = lambda a: np.ascontiguousarray(np.asarray(a), dtype=np.float32)
    x = f(inputs["x"])
    emb = f(inputs["emb"])
    Wih0, Whh0 = f(inputs["Wih0"]), f(inputs["Whh0"])
    bih0, bhh0 = f(inputs["bih0"]), f(inputs["bhh0"])
    Wih1, Whh1 = f(inputs["Wih1"]), f(inputs["Whh1"])
    bih1, bhh1 = f(inputs["bih1"]), f(inputs["bhh1"])
    Wlin, blin = f(inputs["Wlin"]), f(inputs["blin"])

    # Fold embedding + biases into layer-0 input weights.
    w_val = Wih0[:, 0:1]  # [G, 1]
    M0 = Wih0[:, 1 : 1 + D] @ emb.T  # [G, 7]
    b0 = (bih0 + bhh0)[:, None]  # [G, 1]
    W0aug = np.concatenate(
        [w_val, M0, b0, np.zeros((G, 128 - 9), np.float32)], axis=1
    )  # [G, 128]

    w0t = np.ascontiguousarray(W0aug[_PERM].T)  # [128, G]
    whh0t = np.ascontiguousarray(Whh0[_PERM].T)  # [H, G]
    wih1t = np.ascontiguousarray(Wih1[_PERM].T)
    whh1t = np.ascontiguousarray(Whh1[_PERM].T)
    b1f = np.zeros((128, G), np.float32)
    b1f[0] = (bih1 + bhh1)[_PERM]
    e0 = np.zeros((128, 128), np.float32)
    e0[0] = 1.0
    wlint = np.ascontiguousarray(Wlin.T)  # [H, P_OUT]
    blinf = np.zeros((128, P_OUT), np.float32)
    blinf[0] = blin
    z112 = np.zeros((128 - FA, CH * B), np.float32)

    val = x[:, :, 0]  # [B_FULL, T]
    day = x[:, :, 1].astype(np.int32)  # [B_FULL, T]

    in_maps = []
    for c in range(N_CORES):
        sl = slice(c * B, (c + 1) * B)
        aug = np.zeros((FA, T, B), np.float32)
        aug[0] = val[sl].T
        dT = day[sl].T  # [T, B]
        for d in range(7):
            aug[1 + d] = dT == d
        aug[8] = 1.0
        in_maps.append(
            {
                "aug": np.ascontiguousarray(aug.reshape(FA, T * B)),
                "z112": z112,
                "w0t": w0t,
                "whh0t": whh0t,
                "wih1t": wih1t,
                "whh1t": whh1t,
                "e0": e0,
                "b1f": b1f,
                "wlint": wlint,
                "blinf": blinf,
            }
        )

    res = run_bass_kernel_spmd(_get_module(), in_maps, core_ids=list(range(N_CORES)))
    LAST_RESULTS = res
    out = np.concatenate([r["out"] for r in res.results], axis=0)
    return np.ascontiguousarray(out, dtype=np.float32)
